# revision 22
# baseline (speedup 1.0000x reference)
"""GAT (3-layer, 3-head) GNN forward on 8 Trainium2 NeuronCores.

Strategy (v2, f16):
- Host partitions the 64 graphs onto 8 cores (8 graphs each). Node slots are
  padded per graph to a uniform stride so the SPMD program is identical on
  every core.
- Heavy data path in f16: x, weights, h-tables, gathered rows, one-hot
  matrices, scatter matmuls. PSUM accumulation stays fp32.
- Per layer: phase-1 matmul produces node-major rows [h | es | ed] into a
  DRAM table (L1 replicated since x is an input; L2/L3 local + AllGather).
- Aggregation per 128-dst chunk: dma_gather of h[src] rows (f16);
  host-precomputed one-hot matrices o01 [edge,dst] / oT [dst,edge] (shared
  across layers); batched score math (es+ed, leaky, exp) per chunk; per-dst
  1/sum(exp) mapped back to edges via oT matmul; weighted scatter-add via
  f16 PE matmuls accumulating in fp32 PSUM; Relu straight out of PSUM.
- BN stats via AllReduce; head: pooling via fixed-stride reduces + small
  fp32 matmuls.
"""

import os
import numpy as np

H = 3
NEG_SLOPE = 0.2
BN_EPS = 1e-5
NCORES = 8
F16 = np.float16


def _ceil_to(x, m):
    return ((x + m - 1) // m) * m


def _prep(inputs):
    """All host-side (numpy) preprocessing: partitioning, index arrays,
    combined weight tables, one-hot edge matrices."""
    P = {}
    x = np.asarray(inputs["x"], np.float32)
    ei = np.asarray(inputs["edge_index"])
    batch = np.asarray(inputs["batch"]).astype(np.int64)
    N, F_IN = x.shape
    G = int(batch.max()) + 1
    assert G % NCORES == 0
    GPC = G // NCORES  # graphs per core

    counts = np.bincount(batch, minlength=G)
    gstart = np.concatenate([[0], np.cumsum(counts)[:-1]])
    GSTRIDE = _ceil_to(int(counts.max()), max(16, 128 // GPC))
    PAD_N = GPC * GSTRIDE
    assert PAD_N % 128 == 0
    R = NCORES * PAD_N  # total table rows
    assert R < 32768, f"table rows {R} exceed int16 range"
    CHUNKS = PAD_N // 128

    n = np.arange(N)
    g = batch
    slot = (g // GPC) * PAD_N + (g % GPC) * GSTRIDE + (n - gstart[g])
    P.update(N=N, G=G, GPC=GPC, GSTRIDE=GSTRIDE, PAD_N=PAD_N, R=R,
             CHUNKS=CHUNKS, F_IN=F_IN, slot=slot)

    # --- edges with self loops, partitioned by dst core, sorted by dst slot
    src = np.concatenate([ei[0], n]).astype(np.int64)
    dst = np.concatenate([ei[1], n]).astype(np.int64)
    sslot = slot[src]
    dslot = slot[dst]
    dcore = dslot // PAD_N

    per_core = []
    for c in range(NCORES):
        m = dcore == c
        ss, dd = sslot[m], dslot[m] - c * PAD_N
        order = np.argsort(dd, kind="stable")
        per_core.append((ss[order], (dd[order]) % 128, dd[order] // 128))

    tiles_per_chunk = np.zeros(CHUNKS, np.int64)
    for c in range(NCORES):
        _, _, ck = per_core[c]
        cnt = np.bincount(ck, minlength=CHUNKS)
        tiles_per_chunk = np.maximum(tiles_per_chunk, (cnt + 127) // 128)
    TILES = int(tiles_per_chunk.sum())
    tile_base = np.concatenate([[0], np.cumsum(tiles_per_chunk)[:-1]])
    P["tiles_per_chunk"] = tiles_per_chunk
    P["tile_base"] = tile_base
    P["TILES"] = TILES

    # padded per-core arrays: src table ids (dummy 0), dst offsets (-1)
    src_pad = np.zeros((NCORES, TILES * 128), np.int16)
    dst_pad = np.full((NCORES, TILES * 128), -1, np.int64)
    for c in range(NCORES):
        ss, doff, ck = per_core[c]
        cnt = np.bincount(ck, minlength=CHUNKS)
        off = np.concatenate([[0], np.cumsum(cnt)[:-1]])
        for k in range(CHUNKS):
            nk = int(cnt[k])
            if nk == 0:
                continue
            b = int(tile_base[k]) * 128
            src_pad[c, b:b + nk] = ss[off[k]:off[k] + nk]
            dst_pad[c, b:b + nk] = doff[off[k]:off[k] + nk]

    # wrap src ids for dma_gather: idx i of a chunk-gather at [i%16, i//16],
    # columns grouped per chunk; replicated to 128 partitions.
    IDXC = TILES * 8
    src16 = np.zeros((NCORES, 16, IDXC), np.int16)
    for c in range(NCORES):
        for k in range(CHUNKS):
            t0, nt = int(tile_base[k]), int(tiles_per_chunk[k])
            if nt == 0:
                continue
            seg = src_pad[c, t0 * 128:(t0 + nt) * 128]
            src16[c, :, t0 * 8:(t0 + nt) * 8] = seg.reshape(-1, 16).T
    P["src16"] = np.ascontiguousarray(np.tile(src16, (1, 8, 1)))
    P["IDXC"] = IDXC

    # one-hot edge->dst matrices per tile, shared by all 3 layers:
    #   o01[e, t*128+d] = (dst_pad[t*128+e] == d)   (edge-partition)
    #   oT [d, t*128+e] = same                      (dst-partition)
    d_idx = np.arange(128, dtype=np.int64)
    o01 = np.zeros((NCORES, 128, TILES * 128), F16)
    oT = np.zeros((NCORES, 128, TILES * 128), F16)
    for c in range(NCORES):
        dp = dst_pad[c].reshape(TILES, 128)
        blk = (dp[:, :, None] == d_idx[None, None, :])  # [T, e, d]
        o01[c] = blk.transpose(1, 0, 2).reshape(128, TILES * 128).astype(F16)
        oT[c] = blk.transpose(2, 0, 1).reshape(128, TILES * 128).astype(F16)
    P["o01"] = o01
    P["oT"] = oT

    # --- x^T with ones row, slotted, sharded per core (f16)
    xT = np.zeros((F_IN + 1, R), np.float32)
    xT[:F_IN, slot] = x.T
    xT[F_IN, slot] = 1.0
    P["x1T"] = np.ascontiguousarray(
        xT.astype(F16).reshape(F_IN + 1, NCORES, PAD_N).transpose(1, 0, 2))

    # --- combined weight tables [K+1, C*H+6] (f16)
    def comb(W, a_s, a_d, b, C):
        K = W.shape[0]
        Wc = np.zeros((K + 1, H * C + 6), np.float32)
        Wc[:K, :H * C] = W
        for j in range(H):
            Wc[:K, H * C + j] = W[:, j * C:(j + 1) * C] @ a_s[j]
            Wc[:K, H * C + 3 + j] = W[:, j * C:(j + 1) * C] @ a_d[j]
        Wc[K, :H * C] = b
        return Wc.astype(F16)

    P["W1c"] = comb(np.asarray(inputs["W1"], np.float32),
                    np.asarray(inputs["a1_src"], np.float32),
                    np.asarray(inputs["a1_dst"], np.float32),
                    np.asarray(inputs["b1"], np.float32), 256)
    P["W2c"] = comb(np.asarray(inputs["W2"], np.float32),
                    np.asarray(inputs["a2_src"], np.float32),
                    np.asarray(inputs["a2_dst"], np.float32),
                    np.asarray(inputs["b2"], np.float32), 128)
    P["W3c"] = comb(np.asarray(inputs["W3"], np.float32),
                    np.asarray(inputs["a3_src"], np.float32),
                    np.asarray(inputs["a3_dst"], np.float32),
                    np.asarray(inputs["b3"], np.float32), 128)

    def bnpack(gv, bv, nblk):
        t = np.zeros((128, 2 * nblk), np.float32)
        gv = np.asarray(gv, np.float32)
        bv = np.asarray(bv, np.float32)
        for b in range(nblk):
            sg = gv[b * 128:(b + 1) * 128]
            t[:len(sg), b] = sg
            sb = bv[b * 128:(b + 1) * 128]
            t[:len(sb), nblk + b] = sb
        return t

    P["bn1"] = bnpack(inputs["g1"], inputs["be1"], 6)
    P["bn2"] = bnpack(inputs["g2"], inputs["be2"], 3)
    P["bn3"] = bnpack(inputs["g3"], inputs["be3"], 3)
    bn4 = np.zeros((64, 2), np.float32)
    bn4[:, 0] = np.asarray(inputs["g4"], np.float32)
    bn4[:, 1] = np.asarray(inputs["be4"], np.float32)
    P["bn4"] = bn4

    P["fc1w"] = np.asarray(inputs["fc1_w"], np.float32)
    P["fc2w"] = np.asarray(inputs["fc2_w"], np.float32)
    P["fc3w"] = np.asarray(inputs["fc3_w"], np.float32)
    fcb = np.zeros((64, 3), np.float32)
    fcb[:, 0] = np.asarray(inputs["fc1_b"], np.float32)
    fcb[:, 1] = np.asarray(inputs["fc2_b"], np.float32)
    fcb[0, 2] = float(np.asarray(inputs["fc3_b"], np.float32).reshape(-1)[0])
    P["fcb"] = fcb

    cntb = np.zeros((NCORES, 128, GPC), np.float32)
    for c in range(NCORES):
        cntb[c, :, :] = counts[c * GPC:(c + 1) * GPC][None, :]
    P["cntb"] = cntb
    P["ones1"] = np.ones((1, 128), F16)
    return P


def _build(nc, P, mybir, tile, bass, library_config):
    STAGE = int(os.environ.get("GAT_STAGE", "99"))
    dt = mybir.dt
    f32 = dt.float32
    f16 = dt.float16
    AT = mybir.ActivationFunctionType
    OP = mybir.AluOpType
    AX = mybir.AxisListType
    R, PAD_N, CHUNKS, TILES = P["R"], P["PAD_N"], P["CHUNKS"], P["TILES"]
    IDXC, F_IN = P["IDXC"], P["F_IN"]
    GST, GPC = P["GSTRIDE"], P["GPC"]
    NTT = R // 128
    LNT = PAD_N // 128
    tiles_per_chunk = P["tiles_per_chunk"]
    tile_base = P["tile_base"]
    N_REAL, G = P["N"], P["G"]
    TMAX = int(tiles_per_chunk.max())

    # ---------------- DRAM tensors ----------------
    def ein(name, shape, dtype=f32):
        return nc.dram_tensor(name, list(shape), dtype, kind="ExternalInput").ap()

    x1T_d = ein("x1T", [F_IN + 1, PAD_N], f16)
    W1c_d = ein("W1c", P["W1c"].shape, f16)
    W2c_d = ein("W2c", P["W2c"].shape, f16)
    W3c_d = ein("W3c", P["W3c"].shape, f16)
    src16_d = ein("src16", [128, IDXC], dt.int16)
    o01_d = ein("o01", [128, TILES * 128], f16)
    oT_d = ein("oT", [128, TILES * 128], f16)
    ones1_d = ein("ones1", [1, 128], f16)
    bn1_d = ein("bn1", [128, 12])
    bn2_d = ein("bn2", [128, 6])
    bn3_d = ein("bn3", [128, 6])
    bn4_d = ein("bn4", [64, 2])
    fc1w_d = ein("fc1w", [384, 64])
    fc2w_d = ein("fc2w", [64, 64])
    fc3w_d = ein("fc3w", [64, 1])
    fcb_d = ein("fcb", [64, 3])
    cntb_d = ein("cntb", [128, GPC])
    y_d = nc.dram_tensor("y", [1, GPC], f32, kind="ExternalOutput").ap()

    CO1, CO23 = 774, 390            # written row widths (h | es | ed)
    ROW1, ROW23 = 896, 512          # table strides (f16, 256B multiple)
    h1_loc = nc.dram_tensor("h1_loc", [PAD_N, ROW1], f16).ap()
    h1_full = nc.dram_tensor("h1_full", [R, ROW1], f16, addr_space="Shared").ap()
    h2_loc = nc.dram_tensor("h2_loc", [PAD_N, ROW23], f16).ap()
    h2_full = nc.dram_tensor("h2_full", [R, ROW23], f16, addr_space="Shared").ap()
    h3_loc = nc.dram_tensor("h3_loc", [PAD_N, ROW23], f16).ap()
    h3_full = nc.dram_tensor("h3_full", [R, ROW23], f16, addr_space="Shared").ap()
    st_loc = [nc.dram_tensor(f"st{l}_loc", [128, 12], f32).ap() for l in range(3)]
    st_full = [nc.dram_tensor(f"st{l}_full", [128, 12], f32,
                              addr_space="Shared").ap() for l in range(3)]
    st4_loc = nc.dram_tensor("st4_loc", [64, 2], f32).ap()
    st4_full = nc.dram_tensor("st4_full", [64, 2], f32, addr_space="Shared").ap()

    with tile.TileContext(nc) as tc:
        nc.gpsimd.load_library(library_config.mlp)

        with tc.tile_pool(name="pers", bufs=1) as pers:
            ones1 = pers.tile([1, 128], f16, tag="ones1")
            src16 = pers.tile([128, IDXC], dt.int16, tag="src16")
            nc.sync.dma_start(out=ones1[:], in_=ones1_d[:])
            nc.sync.dma_start(out=src16[:], in_=src16_d[:])
            esed1 = pers.tile([128, LNT * 6], f32, tag="esed1")
            esed23 = pers.tile([128, LNT * 6], f32, tag="esed23")

            # =====================================================
            def phase1(xT_tiles, kdims, Wc_d, Wc_rows, CO, tbl, ntiles,
                       esed_sb, with_ones, tagp):
                nK = len(kdims)
                with tc.tile_pool(name=f"w{tagp}", bufs=1) as wp, \
                     tc.tile_pool(name=f"p1{tagp}", bufs=4) as sp, \
                     tc.tile_pool(name=f"ps{tagp}", bufs=3, space="PSUM") as pp, \
                     tc.tile_pool(name=f"x{tagp}", bufs=3) as xp:
                    Wts = []
                    r0 = 0
                    for ki, kd in enumerate(kdims):
                        wt = wp.tile([kd, CO], f16, tag=f"w{ki}")
                        nc.sync.dma_start(out=wt[:], in_=Wc_d[r0:r0 + kd, :])
                        Wts.append(wt)
                        r0 += kd
                    wbot = None
                    if with_ones:
                        wbot = wp.tile([1, CO], f16, tag="wbot")
                        nc.sync.dma_start(out=wbot[:],
                                          in_=Wc_d[Wc_rows - 1:Wc_rows, :])

                    GRP = 8
                    xg = None
                    for nt in range(ntiles):
                        if xT_tiles is None:
                            if nt % GRP == 0:
                                xg = []
                                r0 = 0
                                w = min(GRP * 128, (ntiles - nt) * 128)
                                for ki, kd in enumerate(kdims):
                                    t = xp.tile([kd, GRP * 128], f16,
                                                tag=f"xg{ki}")
                                    nc.sync.dma_start(
                                        out=t[:, :w],
                                        in_=x1T_d[r0:r0 + kd,
                                                  nt * 128:nt * 128 + w])
                                    xg.append(t)
                                    r0 += kd
                            o = (nt % GRP) * 128
                            lhs = [xg[ki][:, o:o + 128] for ki in range(nK)]
                        else:
                            lhs = [xT_tiles[ki][:, nt * 128:(nt + 1) * 128]
                                   for ki in range(nK)]
                        hp = pp.tile([128, CO], f32, tag="hp")
                        n_mm = (CO + 511) // 512
                        for ki in range(nK):
                            for mi in range(n_mm):
                                c0, c1 = mi * 512, min(CO, mi * 512 + 512)
                                nc.tensor.matmul(
                                    hp[:, c0:c1], lhs[ki], Wts[ki][:, c0:c1],
                                    start=(ki == 0),
                                    stop=(not with_ones and ki == nK - 1))
                        if with_ones:
                            for mi in range(n_mm):
                                c0, c1 = mi * 512, min(CO, mi * 512 + 512)
                                nc.tensor.matmul(
                                    hp[:, c0:c1], ones1[:], wbot[:, c0:c1],
                                    start=False, stop=True)
                        COH = CO - 6
                        hs = sp.tile([128, COH + 12], f16, tag="hs")
                        if nt % 2 == 0:
                            nc.vector.tensor_copy(hs[:, :COH], hp[:, :COH])
                        else:
                            nc.scalar.copy(hs[:, :COH], hp[:, :COH])
                        nc.vector.tensor_copy(
                            hs[:, COH:COH + 12].bitcast(f32), hp[:, COH:CO])
                        nc.vector.tensor_copy(esed_sb[:, nt * 6:(nt + 1) * 6],
                                              hp[:, CO - 6:CO])
                        eng = nc.sync if nt % 2 == 0 else nc.gpsimd
                        eng.dma_start(
                            out=tbl[nt * 128:(nt + 1) * 128, 0:COH + 12],
                            in_=hs[:])

            # =====================================================
            def aggregate(tbl, ROW, C, esed_sb, esed_dyn, yT, tagp):
                NB = (H * C) // 128
                nbh = C // 128
                with tc.tile_pool(name=f"g{tagp}", bufs=5) as gp, \
                     tc.tile_pool(name=f"o{tagp}", bufs=5) as op, \
                     tc.tile_pool(name=f"a{tagp}", bufs=5) as ap, \
                     tc.tile_pool(name=f"q{tagp}", bufs=2, space="PSUM") as qa, \
                     tc.tile_pool(name=f"e{tagp}", bufs=2, space="PSUM") as qb, \
                     tc.tile_pool(name=f"s{tagp}", bufs=2, space="PSUM") as qc:
                    for b in range(NB):
                        nc.vector.memset(yT[b][:], 0.0)
                    APAD = 1024 if NB == 6 else 512
                    for k in range(CHUNKS):
                        T = int(tiles_per_chunk[k])
                        if T == 0:
                            continue
                        t0 = int(tile_base[k])
                        edch = ap.tile([128, 6], f32, tag="edch")
                        nc.vector.tensor_copy(
                            edch[:], esed_sb[:, k * 6:k * 6 + 6])
                        hg = gp.tile([128, TMAX, ROW], f16, tag="hg")
                        nidx = T * 128
                        nc.gpsimd.dma_gather(
                            hg[:, :T, :], tbl[:], src16[:, t0 * 8:(t0 + T) * 8],
                            nidx, nidx, ROW, single_packet=False)
                        o01t = op.tile([128, TMAX * 128], f16, tag="o01t")
                        oTt = op.tile([128, TMAX * 128], f16, tag="oTt")
                        nc.sync.dma_start(
                            out=o01t[:, :T * 128],
                            in_=o01_d[:, t0 * 128:(t0 + T) * 128])
                        nc.sync.dma_start(
                            out=oTt[:, :T * 128],
                            in_=oT_d[:, t0 * 128:(t0 + T) * 128])

                        # pass 1: ed[dst] per edge -> scores -> exp.
                        # f32 values ride through f16 PE selections as
                        # hi/lo f16 pairs (exact to ~16 mantissa bits).
                        edhl = ap.tile([128, 3], f16, tag="edhl")
                        nc.vector.tensor_copy(edhl[:], edch[:, 3:6])
                        edpA = qb.tile([128, TMAX, 3], f32, tag="edpA")
                        for t in range(T):
                            nc.tensor.matmul(
                                edpA[:, t, :], oTt[:, t * 128:(t + 1) * 128],
                                edhl[:], start=True, stop=True)
                        exf = ap.tile([128, TMAX, 3], f32, tag="exf")
                        nc.vector.tensor_tensor(
                            exf[:, :T, :],
                            hg[:, :T, H * C:H * C + 12].bitcast(f32)[:, :, 0:3],
                            edpA[:, :T, :], OP.add)
                        exm = ap.tile([128, TMAX, 3], f32, tag="exm")
                        nc.vector.tensor_scalar_mul(exm[:, :T, :],
                                                    exf[:, :T, :], NEG_SLOPE)
                        nc.vector.tensor_tensor(exf[:, :T, :], exf[:, :T, :],
                                                exm[:, :T, :], OP.max)
                        nc.vector.tensor_scalar_add(exf[:, :T, :],
                                                    exf[:, :T, :], -5.0)
                        expf = ap.tile([128, TMAX, 3], f32, tag="expf")
                        nc.scalar.activation(expf[:, :T, :], exf[:, :T, :],
                                             AT.Exp)
                        exb = ap.tile([128, TMAX, 3], f16, tag="exb")
                        nc.vector.tensor_copy(exb[:, :T, :], expf[:, :T, :])

                        # pass 2: per-dst sums -> reciprocal -> back to edges
                        spT = qc.tile([128, 4], f32, tag="spT")
                        for t in range(T):
                            nc.tensor.matmul(
                                spT[:, 0:3], o01t[:, t * 128:(t + 1) * 128],
                                exb[:, t, :], start=(t == 0), stop=(t == T - 1))
                        srT = ap.tile([128, 3], f32, tag="srT")
                        nc.vector.tensor_copy(srT[:], spT[:, 0:3])
                        nc.vector.tensor_scalar_add(srT[:], srT[:], 1e-16)
                        nc.vector.reciprocal(srT[:], srT[:])
                        nc.vector.tensor_scalar_min(srT[:], srT[:], 60000.0)
                        srhl = ap.tile([128, 3], f16, tag="srhl")
                        nc.vector.tensor_copy(srhl[:], srT[:])
                        srbA = qb.tile([128, TMAX, 3], f32, tag="edpA")
                        for t in range(T):
                            nc.tensor.matmul(
                                srbA[:, t, :], oTt[:, t * 128:(t + 1) * 128],
                                srhl[:], start=True, stop=True)
                        exsc = ap.tile([128, TMAX, 3], f32, tag="exsc")
                        nc.vector.tensor_tensor(exsc[:, :T, :], expf[:, :T, :],
                                                srbA[:, :T, :], OP.mult)

                        # pass 3: weighted scatter-add via PE
                        aggp = qa.tile([128, APAD], f32, tag="aggp")
                        for t in range(T):
                            ows = []
                            for h in range(H):
                                ow = ap.tile([128, 128], f16, tag=f"ow{h}")
                                if h == 0:
                                    nc.scalar.activation(
                                        ow[:], o01t[:, t * 128:(t + 1) * 128],
                                        AT.Identity,
                                        scale=exsc[:, t, h:h + 1])
                                else:
                                    nc.vector.tensor_scalar(
                                        ow[:], o01t[:, t * 128:(t + 1) * 128],
                                        exsc[:, t, h:h + 1], None, OP.mult)
                                ows.append(ow)
                            for fb in range(NB):
                                h = fb // nbh
                                st = (t == 0) and (fb % 4 == 0)
                                sp2 = (t == T - 1) and (
                                    fb % 4 == 3 or fb == NB - 1)
                                nc.tensor.matmul(
                                    aggp[:, fb * 128:(fb + 1) * 128],
                                    hg[:, t,
                                       h * C + (fb % nbh) * 128:
                                       h * C + (fb % nbh) * 128 + 128],
                                    ows[h][:],
                                    start=st, stop=sp2)
                        # epilogue: relu straight out of PSUM
                        for fb in range(NB):
                            if fb % 2 == 0:
                                nc.scalar.activation(
                                    yT[fb][:, k * 128:(k + 1) * 128],
                                    aggp[:, fb * 128:(fb + 1) * 128], AT.Relu)
                            else:
                                nc.vector.tensor_scalar_max(
                                    yT[fb][:, k * 128:(k + 1) * 128],
                                    aggp[:, fb * 128:(fb + 1) * 128], 0.0)

            # =====================================================
            def bn(yT, NB, bn_d, stl, stf, tagp, apply=True, sc_out=None,
                   sh_out=None):
                with tc.tile_pool(name=f"b{tagp}", bufs=1) as bp:
                    scratch = bp.tile([128, PAD_N], f16, tag="scr")
                    stats = bp.tile([128, 12], f32, tag="stats")
                    nc.vector.memset(stats[:], 0.0)
                    for b in range(NB):
                        nc.vector.tensor_reduce(stats[:, b:b + 1], yT[b][:],
                                                AX.X, OP.add)
                        nc.vector.tensor_tensor(scratch[:], yT[b][:],
                                                yT[b][:], OP.mult)
                        nc.vector.tensor_reduce(stats[:, 6 + b:7 + b],
                                                scratch[:], AX.X, OP.add)
                    nc.sync.dma_start(out=stl[:], in_=stats[:])
                    nc.gpsimd.collective_compute(
                        "AllReduce", OP.add,
                        replica_groups=[list(range(NCORES))],
                        ins=[stl[:]], outs=[stf[:]])
                    stg = bp.tile([128, 12], f32, tag="stg")
                    nc.sync.dma_start(out=stg[:], in_=stf[:])
                    bnp = bp.tile([128, 2 * NB], f32, tag="bnp")
                    nc.sync.dma_start(out=bnp[:], in_=bn_d[:])
                    mu = bp.tile([128, 6], f32, tag="mu")
                    var = bp.tile([128, 6], f32, tag="var")
                    sc = bp.tile([128, 6], f32, tag="sc")
                    sh = bp.tile([128, 6], f32, tag="sh")
                    inv_n = 1.0 / float(N_REAL)
                    nc.vector.tensor_scalar_mul(mu[:, :NB], stg[:, :NB], inv_n)
                    nc.vector.tensor_scalar_mul(var[:, :NB], stg[:, 6:6 + NB],
                                                inv_n)
                    nc.vector.tensor_tensor(sc[:, :NB], mu[:, :NB], mu[:, :NB],
                                            OP.mult)
                    nc.vector.tensor_tensor(var[:, :NB], var[:, :NB],
                                            sc[:, :NB], OP.subtract)
                    nc.vector.tensor_scalar_add(var[:, :NB], var[:, :NB],
                                                BN_EPS)
                    nc.scalar.activation(var[:, :NB], var[:, :NB], AT.Sqrt)
                    nc.vector.reciprocal(var[:, :NB], var[:, :NB])
                    nc.vector.tensor_tensor(sc[:, :NB], bnp[:, :NB],
                                            var[:, :NB], OP.mult)
                    nc.vector.tensor_tensor(mu[:, :NB], mu[:, :NB], sc[:, :NB],
                                            OP.mult)
                    nc.vector.tensor_tensor(sh[:, :NB], bnp[:, NB:2 * NB],
                                            mu[:, :NB], OP.subtract)
                    if apply:
                        for b in range(NB):
                            nc.scalar.activation(yT[b][:], yT[b][:],
                                                 AT.Identity,
                                                 bias=sh[:, b:b + 1],
                                                 scale=sc[:, b:b + 1])
                    else:
                        nc.vector.tensor_copy(sc_out[:], sc[:, :NB])
                        nc.vector.tensor_copy(sh_out[:], sh[:, :NB])

            # ================= Layer 1 =================
            kdims1 = []
            rem = F_IN + 1
            while rem > 0:
                kdims1.append(min(128, rem))
                rem -= kdims1[-1]
            phase1(None, kdims1, W1c_d, F_IN + 1, CO1, h1_loc, LNT, esed1,
                   with_ones=False, tagp="l1")
            nc.gpsimd.collective_compute(
                "AllGather", OP.bypass,
                replica_groups=[list(range(NCORES))],
                ins=[h1_loc[:]], outs=[h1_full[:]])

            def dbg_finish(t128):
                o = pers.tile([1, GPC], f32, tag="dbgy")
                nc.vector.tensor_copy(o[:], t128[0:1, 0:GPC])
                nc.sync.dma_start(out=y_d[:], in_=o[:])

            if STAGE <= 0:
                dbg_finish(esed1)
                return
            with tc.tile_pool(name="y1", bufs=1) as y1p:
                yT1 = [y1p.tile([128, PAD_N], f16, tag=f"y1_{b}", name=f"y1_{b}")
                       for b in range(6)]
                aggregate(h1_full, ROW1, 256, esed1, False, yT1, "l1")
                if STAGE <= 1:
                    dbg_finish(yT1[0])
                    return
                bn(yT1, 6, bn1_d, st_loc[0], st_full[0], "l1")
                if STAGE <= 2:
                    dbg_finish(yT1[0])
                    return

                # ================= Layer 2 =================
                phase1(yT1, [128] * 6, W2c_d, 769, CO23, h2_loc, LNT, esed23,
                       with_ones=True, tagp="l2")
            if STAGE <= 3:
                dbg_finish(esed23)
                return
            nc.gpsimd.collective_compute(
                "AllGather", OP.bypass,
                replica_groups=[list(range(NCORES))],
                ins=[h2_loc[:]], outs=[h2_full[:]])
            if STAGE <= 4:
                dbg_finish(esed23)
                return
            with tc.tile_pool(name="y2", bufs=1) as y2p:
                yT2 = [y2p.tile([128, PAD_N], f16, tag=f"y2_{b}", name=f"y2_{b}")
                       for b in range(3)]
                aggregate(h2_full, ROW23, 128, esed23, False, yT2, "l2")
                if STAGE <= 5:
                    dbg_finish(yT2[0])
                    return
                bn(yT2, 3, bn2_d, st_loc[1], st_full[1], "l2")

                # ================= Layer 3 =================
                phase1(yT2, [128] * 3, W3c_d, 385, CO23, h3_loc, LNT, esed23,
                       with_ones=True, tagp="l3")
            nc.gpsimd.collective_compute(
                "AllGather", OP.bypass,
                replica_groups=[list(range(NCORES))],
                ins=[h3_loc[:]], outs=[h3_full[:]])
            if STAGE <= 7:
                dbg_finish(esed23)
                return
            with tc.tile_pool(name="y3", bufs=1) as y3p:
                yT3 = [y3p.tile([128, PAD_N], f16, tag=f"y3_{b}", name=f"y3_{b}")
                       for b in range(3)]
                # ================= Head =================
                with tc.tile_pool(name="hd", bufs=1) as hp_:
                    sc3 = hp_.tile([128, 3], f32, tag="sc3")
                    sh3 = hp_.tile([128, 3], f32, tag="sh3")
                    aggregate(h3_full, ROW23, 128, esed23, False, yT3, "l3")
                    if STAGE <= 8:
                        dbg_finish(yT3[0])
                        return
                    bn(yT3, 3, bn3_d, st_loc[2], st_full[2], "l3",
                       apply=False, sc_out=sc3, sh_out=sh3)
                    pp_cm = tc.tile_pool(name="hdp", bufs=1, space="PSUM")
                    pp_ = pp_cm.__enter__()
                    cntb = hp_.tile([128, GPC], f32, tag="cntb")
                    nc.sync.dma_start(out=cntb[:], in_=cntb_d[:])
                    poolT = hp_.tile([128, 3 * GPC], f32, tag="poolT")
                    shc = hp_.tile([128, GPC], f32, tag="shc")
                    for b in range(3):
                        for g_ in range(GPC):
                            nc.vector.tensor_reduce(
                                poolT[:, b * GPC + g_:b * GPC + g_ + 1],
                                yT3[b][:, g_ * GST:(g_ + 1) * GST],
                                AX.X, OP.add)
                        # pool(BN(y)) = pool(y)*sc + cnt_g*sh
                        nc.vector.tensor_scalar(
                            poolT[:, b * GPC:(b + 1) * GPC],
                            poolT[:, b * GPC:(b + 1) * GPC],
                            sc3[:, b:b + 1], None, OP.mult)
                        nc.vector.tensor_scalar(
                            shc[:], cntb[:], sh3[:, b:b + 1], None, OP.mult)
                        nc.vector.tensor_tensor(
                            poolT[:, b * GPC:(b + 1) * GPC],
                            poolT[:, b * GPC:(b + 1) * GPC],
                            shc[:], OP.add)
                    fc1w = hp_.tile([128, 3, 64], f32, tag="fc1w")
                    for b in range(3):
                        nc.sync.dma_start(out=fc1w[:, b, :],
                                          in_=fc1w_d[b * 128:(b + 1) * 128, :])
                    fc2w = hp_.tile([64, 64], f32, tag="fc2w")
                    nc.sync.dma_start(out=fc2w[:], in_=fc2w_d[:])
                    fc3w = hp_.tile([64, 1], f32, tag="fc3w")
                    nc.sync.dma_start(out=fc3w[:], in_=fc3w_d[:])
                    fcb = hp_.tile([64, 3], f32, tag="fcb")
                    nc.sync.dma_start(out=fcb[:], in_=fcb_d[:])
                    bn4 = hp_.tile([64, 2], f32, tag="bn4")
                    nc.sync.dma_start(out=bn4[:], in_=bn4_d[:])

                    p1p = pp_.tile([64, GPC], f32, tag="p1p")
                    for b in range(3):
                        nc.tensor.matmul(p1p[:], fc1w[:, b, :],
                                         poolT[:, b * GPC:(b + 1) * GPC],
                                         start=(b == 0), stop=(b == 2))
                    p1 = hp_.tile([64, GPC], f32, tag="p1")
                    nc.scalar.activation(p1[:], p1p[:], AT.Relu,
                                         bias=fcb[:, 0:1])
                    st4 = hp_.tile([64, 2], f32, tag="st4")
                    scr4 = hp_.tile([64, GPC], f32, tag="scr4")
                    nc.vector.tensor_reduce(st4[:, 0:1], p1[:], AX.X, OP.add)
                    nc.vector.tensor_tensor(scr4[:], p1[:], p1[:], OP.mult)
                    nc.vector.tensor_reduce(st4[:, 1:2], scr4[:], AX.X,
                                            OP.add)
                    nc.sync.dma_start(out=st4_loc[:], in_=st4[:])
                    nc.gpsimd.collective_compute(
                        "AllReduce", OP.add,
                        replica_groups=[list(range(NCORES))],
                        ins=[st4_loc[:]], outs=[st4_full[:]])
                    st4g = hp_.tile([64, 2], f32, tag="st4g")
                    nc.sync.dma_start(out=st4g[:], in_=st4_full[:])
                    mu4 = hp_.tile([64, 4], f32, tag="mu4")
                    inv_g = 1.0 / float(G)
                    nc.vector.tensor_scalar_mul(mu4[:, 0:2], st4g[:], inv_g)
                    nc.vector.tensor_tensor(mu4[:, 2:3], mu4[:, 0:1],
                                            mu4[:, 0:1], OP.mult)
                    nc.vector.tensor_tensor(mu4[:, 1:2], mu4[:, 1:2],
                                            mu4[:, 2:3], OP.subtract)
                    nc.vector.tensor_scalar_add(mu4[:, 1:2], mu4[:, 1:2],
                                                BN_EPS)
                    nc.scalar.activation(mu4[:, 1:2], mu4[:, 1:2], AT.Sqrt)
                    nc.vector.reciprocal(mu4[:, 1:2], mu4[:, 1:2])
                    nc.vector.tensor_tensor(mu4[:, 2:3], bn4[:, 0:1],
                                            mu4[:, 1:2], OP.mult)
                    nc.vector.tensor_tensor(mu4[:, 3:4], mu4[:, 0:1],
                                            mu4[:, 2:3], OP.mult)
                    nc.vector.tensor_tensor(mu4[:, 3:4], bn4[:, 1:2],
                                            mu4[:, 3:4], OP.subtract)
                    nc.scalar.activation(p1[:], p1[:], AT.Identity,
                                         bias=mu4[:, 3:4], scale=mu4[:, 2:3])
                    p2p = pp_.tile([64, GPC], f32, tag="p2p")
                    nc.tensor.matmul(p2p[:], fc2w[:], p1[:], start=True,
                                     stop=True)
                    p2 = hp_.tile([64, GPC], f32, tag="p2")
                    nc.scalar.activation(p2[:], p2p[:], AT.Relu,
                                         bias=fcb[:, 1:2])
                    p3p = pp_.tile([1, GPC], f32, tag="p3p")
                    nc.tensor.matmul(p3p[:], fc3w[:], p2[:], start=True,
                                     stop=True)
                    p3 = hp_.tile([1, GPC], f32, tag="p3")
                    nc.scalar.activation(p3[:], p3p[:], AT.Relu,
                                         bias=fcb[0:1, 2:3])
                    nc.sync.dma_start(out=y_d[:], in_=p3[:])
                    pp_cm.__exit__(None, None, None)


def make_in_maps(P):
    in_maps = []
    for c in range(NCORES):
        in_maps.append({
            "x1T": P["x1T"][c], "W1c": P["W1c"], "W2c": P["W2c"], "W3c": P["W3c"],
            "src16": P["src16"][c], "o01": P["o01"][c], "oT": P["oT"][c],
            "ones1": P["ones1"],
            "bn1": P["bn1"], "bn2": P["bn2"], "bn3": P["bn3"], "bn4": P["bn4"],
            "fc1w": P["fc1w"], "fc2w": P["fc2w"], "fc3w": P["fc3w"],
            "fcb": P["fcb"], "cntb": P["cntb"][c],
        })
    return in_maps


def build_program(P):
    from concourse import bass, mybir, tile, bacc, library_config
    nc = bacc.Bacc("TRN2", target_bir_lowering=False, debug=False,
                   num_devices=NCORES)
    _build(nc, P, mybir, tile, bass, library_config)
    nc.compile()
    return nc


def kernel(**inputs):
    from concourse.bass_utils import run_bass_kernel_spmd
    P = _prep(inputs)
    nc = build_program(P)
    res = run_bass_kernel_spmd(nc, make_in_maps(P), list(range(NCORES)))
    out = np.concatenate([res.results[c]["y"][0] for c in range(NCORES)])
    return out.astype(np.float32)


# revision 23
# speedup vs baseline: 1.0219x; 1.0219x over previous
"""GAT (3-layer, 3-head) GNN forward on 8 Trainium2 NeuronCores.

Strategy (v2, f16):
- Host partitions the 64 graphs onto 8 cores (8 graphs each). Node slots are
  padded per graph to a uniform stride so the SPMD program is identical on
  every core.
- Heavy data path in f16: x, weights, h-tables, gathered rows, one-hot
  matrices, scatter matmuls. PSUM accumulation stays fp32.
- Per layer: phase-1 matmul produces node-major rows [h | es | ed] into a
  DRAM table (L1 replicated since x is an input; L2/L3 local + AllGather).
- Aggregation per 128-dst chunk: dma_gather of h[src] rows (f16);
  host-precomputed one-hot matrices o01 [edge,dst] / oT [dst,edge] (shared
  across layers); batched score math (es+ed, leaky, exp) per chunk; per-dst
  1/sum(exp) mapped back to edges via oT matmul; weighted scatter-add via
  f16 PE matmuls accumulating in fp32 PSUM; Relu straight out of PSUM.
- BN stats via AllReduce; head: pooling via fixed-stride reduces + small
  fp32 matmuls.
"""

import os
import numpy as np

H = 3
NEG_SLOPE = 0.2
BN_EPS = 1e-5
NCORES = 8
F16 = np.float16


def _ceil_to(x, m):
    return ((x + m - 1) // m) * m


def _prep(inputs):
    """All host-side (numpy) preprocessing: partitioning, index arrays,
    combined weight tables, one-hot edge matrices."""
    P = {}
    x = np.asarray(inputs["x"], np.float32)
    ei = np.asarray(inputs["edge_index"])
    batch = np.asarray(inputs["batch"]).astype(np.int64)
    N, F_IN = x.shape
    G = int(batch.max()) + 1
    assert G % NCORES == 0
    GPC = G // NCORES  # graphs per core

    counts = np.bincount(batch, minlength=G)
    gstart = np.concatenate([[0], np.cumsum(counts)[:-1]])
    GSTRIDE = _ceil_to(int(counts.max()), max(16, 128 // GPC))
    PAD_N = GPC * GSTRIDE
    assert PAD_N % 128 == 0
    R = NCORES * PAD_N  # total table rows
    assert R < 32768, f"table rows {R} exceed int16 range"
    CHUNKS = PAD_N // 128

    n = np.arange(N)
    g = batch
    slot = (g // GPC) * PAD_N + (g % GPC) * GSTRIDE + (n - gstart[g])
    P.update(N=N, G=G, GPC=GPC, GSTRIDE=GSTRIDE, PAD_N=PAD_N, R=R,
             CHUNKS=CHUNKS, F_IN=F_IN, slot=slot)

    # --- edges with self loops, partitioned by dst core, sorted by dst slot
    src = np.concatenate([ei[0], n]).astype(np.int64)
    dst = np.concatenate([ei[1], n]).astype(np.int64)
    sslot = slot[src]
    dslot = slot[dst]
    dcore = dslot // PAD_N

    per_core = []
    for c in range(NCORES):
        m = dcore == c
        ss, dd = sslot[m], dslot[m] - c * PAD_N
        order = np.argsort(dd, kind="stable")
        per_core.append((ss[order], (dd[order]) % 128, dd[order] // 128))

    tiles_per_chunk = np.zeros(CHUNKS, np.int64)
    for c in range(NCORES):
        _, _, ck = per_core[c]
        cnt = np.bincount(ck, minlength=CHUNKS)
        tiles_per_chunk = np.maximum(tiles_per_chunk, (cnt + 127) // 128)
    TILES = int(tiles_per_chunk.sum())
    tile_base = np.concatenate([[0], np.cumsum(tiles_per_chunk)[:-1]])
    P["tiles_per_chunk"] = tiles_per_chunk
    P["tile_base"] = tile_base
    P["TILES"] = TILES

    # padded per-core arrays: src table ids (dummy 0), dst offsets (-1)
    src_pad = np.zeros((NCORES, TILES * 128), np.int16)
    dst_pad = np.full((NCORES, TILES * 128), -1, np.int64)
    for c in range(NCORES):
        ss, doff, ck = per_core[c]
        cnt = np.bincount(ck, minlength=CHUNKS)
        off = np.concatenate([[0], np.cumsum(cnt)[:-1]])
        for k in range(CHUNKS):
            nk = int(cnt[k])
            if nk == 0:
                continue
            b = int(tile_base[k]) * 128
            src_pad[c, b:b + nk] = ss[off[k]:off[k] + nk]
            dst_pad[c, b:b + nk] = doff[off[k]:off[k] + nk]

    # wrap src ids for dma_gather: idx i of a chunk-gather at [i%16, i//16],
    # columns grouped per chunk; replicated to 128 partitions.
    IDXC = TILES * 8
    src16 = np.zeros((NCORES, 16, IDXC), np.int16)
    for c in range(NCORES):
        for k in range(CHUNKS):
            t0, nt = int(tile_base[k]), int(tiles_per_chunk[k])
            if nt == 0:
                continue
            seg = src_pad[c, t0 * 128:(t0 + nt) * 128]
            src16[c, :, t0 * 8:(t0 + nt) * 8] = seg.reshape(-1, 16).T
    P["src16"] = np.ascontiguousarray(np.tile(src16, (1, 8, 1)))
    P["IDXC"] = IDXC

    # one-hot edge->dst matrices per tile, shared by all 3 layers:
    #   o01[e, t*128+d] = (dst_pad[t*128+e] == d)   (edge-partition)
    #   oT [d, t*128+e] = same                      (dst-partition)
    d_idx = np.arange(128, dtype=np.int64)
    o01 = np.zeros((NCORES, 128, TILES * 128), F16)
    oT = np.zeros((NCORES, 128, TILES * 128), F16)
    for c in range(NCORES):
        dp = dst_pad[c].reshape(TILES, 128)
        blk = (dp[:, :, None] == d_idx[None, None, :])  # [T, e, d]
        o01[c] = blk.transpose(1, 0, 2).reshape(128, TILES * 128).astype(F16)
        oT[c] = blk.transpose(2, 0, 1).reshape(128, TILES * 128).astype(F16)
    P["o01"] = o01
    P["oT"] = oT

    # --- x^T with ones row, slotted, sharded per core (f16)
    xT = np.zeros((F_IN + 1, R), np.float32)
    xT[:F_IN, slot] = x.T
    xT[F_IN, slot] = 1.0
    P["x1T"] = np.ascontiguousarray(
        xT.astype(F16).reshape(F_IN + 1, NCORES, PAD_N).transpose(1, 0, 2))

    # --- combined weight tables [K+1, C*H+6] (f16)
    def comb(W, a_s, a_d, b, C):
        K = W.shape[0]
        Wc = np.zeros((K + 1, H * C + 6), np.float32)
        Wc[:K, :H * C] = W
        for j in range(H):
            Wc[:K, H * C + j] = W[:, j * C:(j + 1) * C] @ a_s[j]
            Wc[:K, H * C + 3 + j] = W[:, j * C:(j + 1) * C] @ a_d[j]
        Wc[K, :H * C] = b
        return Wc.astype(F16)

    P["W1c"] = comb(np.asarray(inputs["W1"], np.float32),
                    np.asarray(inputs["a1_src"], np.float32),
                    np.asarray(inputs["a1_dst"], np.float32),
                    np.asarray(inputs["b1"], np.float32), 256)
    P["W2c"] = comb(np.asarray(inputs["W2"], np.float32),
                    np.asarray(inputs["a2_src"], np.float32),
                    np.asarray(inputs["a2_dst"], np.float32),
                    np.asarray(inputs["b2"], np.float32), 128)
    P["W3c"] = comb(np.asarray(inputs["W3"], np.float32),
                    np.asarray(inputs["a3_src"], np.float32),
                    np.asarray(inputs["a3_dst"], np.float32),
                    np.asarray(inputs["b3"], np.float32), 128)

    def bnpack(gv, bv, nblk):
        t = np.zeros((128, 2 * nblk), np.float32)
        gv = np.asarray(gv, np.float32)
        bv = np.asarray(bv, np.float32)
        for b in range(nblk):
            sg = gv[b * 128:(b + 1) * 128]
            t[:len(sg), b] = sg
            sb = bv[b * 128:(b + 1) * 128]
            t[:len(sb), nblk + b] = sb
        return t

    P["bn1"] = bnpack(inputs["g1"], inputs["be1"], 6)
    P["bn2"] = bnpack(inputs["g2"], inputs["be2"], 3)
    P["bn3"] = bnpack(inputs["g3"], inputs["be3"], 3)
    bn4 = np.zeros((64, 2), np.float32)
    bn4[:, 0] = np.asarray(inputs["g4"], np.float32)
    bn4[:, 1] = np.asarray(inputs["be4"], np.float32)
    P["bn4"] = bn4

    P["fc1w"] = np.asarray(inputs["fc1_w"], np.float32)
    P["fc2w"] = np.asarray(inputs["fc2_w"], np.float32)
    P["fc3w"] = np.asarray(inputs["fc3_w"], np.float32)
    fcb = np.zeros((64, 3), np.float32)
    fcb[:, 0] = np.asarray(inputs["fc1_b"], np.float32)
    fcb[:, 1] = np.asarray(inputs["fc2_b"], np.float32)
    fcb[0, 2] = float(np.asarray(inputs["fc3_b"], np.float32).reshape(-1)[0])
    P["fcb"] = fcb

    cntb = np.zeros((NCORES, 128, GPC), np.float32)
    for c in range(NCORES):
        cntb[c, :, :] = counts[c * GPC:(c + 1) * GPC][None, :]
    P["cntb"] = cntb
    P["ones1"] = np.ones((1, 128), F16)
    return P


def _build(nc, P, mybir, tile, bass, library_config):
    STAGE = int(os.environ.get("GAT_STAGE", "99"))
    dt = mybir.dt
    f32 = dt.float32
    f16 = dt.float16
    AT = mybir.ActivationFunctionType
    OP = mybir.AluOpType
    AX = mybir.AxisListType
    R, PAD_N, CHUNKS, TILES = P["R"], P["PAD_N"], P["CHUNKS"], P["TILES"]
    IDXC, F_IN = P["IDXC"], P["F_IN"]
    GST, GPC = P["GSTRIDE"], P["GPC"]
    NTT = R // 128
    LNT = PAD_N // 128
    tiles_per_chunk = P["tiles_per_chunk"]
    tile_base = P["tile_base"]
    N_REAL, G = P["N"], P["G"]
    TMAX = int(tiles_per_chunk.max())

    # ---------------- DRAM tensors ----------------
    def ein(name, shape, dtype=f32):
        return nc.dram_tensor(name, list(shape), dtype, kind="ExternalInput").ap()

    x1T_d = ein("x1T", [F_IN + 1, PAD_N], f16)
    W1c_d = ein("W1c", P["W1c"].shape, f16)
    W2c_d = ein("W2c", P["W2c"].shape, f16)
    W3c_d = ein("W3c", P["W3c"].shape, f16)
    src16_d = ein("src16", [128, IDXC], dt.int16)
    o01_d = ein("o01", [128, TILES * 128], f16)
    oT_d = ein("oT", [128, TILES * 128], f16)
    ones1_d = ein("ones1", [1, 128], f16)
    bn1_d = ein("bn1", [128, 12])
    bn2_d = ein("bn2", [128, 6])
    bn3_d = ein("bn3", [128, 6])
    bn4_d = ein("bn4", [64, 2])
    fc1w_d = ein("fc1w", [384, 64])
    fc2w_d = ein("fc2w", [64, 64])
    fc3w_d = ein("fc3w", [64, 1])
    fcb_d = ein("fcb", [64, 3])
    cntb_d = ein("cntb", [128, GPC])
    y_d = nc.dram_tensor("y", [1, GPC], f32, kind="ExternalOutput").ap()

    CO1, CO23 = 774, 390            # written row widths (h | es | ed)
    ROW1, ROW23 = 896, 512          # table strides (f16, 256B multiple)
    h1_loc = nc.dram_tensor("h1_loc", [PAD_N, ROW1], f16).ap()
    h1_full = nc.dram_tensor("h1_full", [R, ROW1], f16, addr_space="Shared").ap()
    h2_loc = nc.dram_tensor("h2_loc", [PAD_N, ROW23], f16).ap()
    h2_full = nc.dram_tensor("h2_full", [R, ROW23], f16, addr_space="Shared").ap()
    h3_loc = nc.dram_tensor("h3_loc", [PAD_N, ROW23], f16).ap()
    h3_full = nc.dram_tensor("h3_full", [R, ROW23], f16, addr_space="Shared").ap()
    st_loc = [nc.dram_tensor(f"st{l}_loc", [128, 12], f32).ap() for l in range(3)]
    st_full = [nc.dram_tensor(f"st{l}_full", [128, 12], f32,
                              addr_space="Shared").ap() for l in range(3)]
    st4_loc = nc.dram_tensor("st4_loc", [64, 2], f32).ap()
    st4_full = nc.dram_tensor("st4_full", [64, 2], f32, addr_space="Shared").ap()

    with tile.TileContext(nc) as tc:
        nc.gpsimd.load_library(library_config.mlp)

        with tc.tile_pool(name="pers", bufs=1) as pers:
            ones1 = pers.tile([1, 128], f16, tag="ones1")
            src16 = pers.tile([128, IDXC], dt.int16, tag="src16")
            nc.sync.dma_start(out=ones1[:], in_=ones1_d[:])
            nc.sync.dma_start(out=src16[:], in_=src16_d[:])
            esed1 = pers.tile([128, LNT * 6], f32, tag="esed1")
            esed23 = pers.tile([128, LNT * 6], f32, tag="esed23")

            # =====================================================
            def phase1(xT_tiles, kdims, Wc_d, Wc_rows, CO, tbl, ntiles,
                       esed_sb, with_ones, tagp):
                nK = len(kdims)
                with tc.tile_pool(name=f"w{tagp}", bufs=1) as wp, \
                     tc.tile_pool(name=f"p1{tagp}", bufs=4) as sp, \
                     tc.tile_pool(name=f"ps{tagp}", bufs=3, space="PSUM") as pp, \
                     tc.tile_pool(name=f"x{tagp}", bufs=3) as xp:
                    Wts = []
                    r0 = 0
                    for ki, kd in enumerate(kdims):
                        wt = wp.tile([kd, CO], f16, tag=f"w{ki}")
                        nc.sync.dma_start(out=wt[:], in_=Wc_d[r0:r0 + kd, :])
                        Wts.append(wt)
                        r0 += kd
                    wbot = None
                    if with_ones:
                        wbot = wp.tile([1, CO], f16, tag="wbot")
                        nc.sync.dma_start(out=wbot[:],
                                          in_=Wc_d[Wc_rows - 1:Wc_rows, :])

                    GRP = 8
                    xg = None
                    for nt in range(ntiles):
                        if xT_tiles is None:
                            if nt % GRP == 0:
                                xg = []
                                r0 = 0
                                w = min(GRP * 128, (ntiles - nt) * 128)
                                for ki, kd in enumerate(kdims):
                                    t = xp.tile([kd, GRP * 128], f16,
                                                tag=f"xg{ki}")
                                    nc.sync.dma_start(
                                        out=t[:, :w],
                                        in_=x1T_d[r0:r0 + kd,
                                                  nt * 128:nt * 128 + w])
                                    xg.append(t)
                                    r0 += kd
                            o = (nt % GRP) * 128
                            lhs = [xg[ki][:, o:o + 128] for ki in range(nK)]
                        else:
                            lhs = [xT_tiles[ki][:, nt * 128:(nt + 1) * 128]
                                   for ki in range(nK)]
                        hp = pp.tile([128, CO], f32, tag="hp")
                        n_mm = (CO + 511) // 512
                        for ki in range(nK):
                            for mi in range(n_mm):
                                c0, c1 = mi * 512, min(CO, mi * 512 + 512)
                                nc.tensor.matmul(
                                    hp[:, c0:c1], lhs[ki], Wts[ki][:, c0:c1],
                                    start=(ki == 0),
                                    stop=(not with_ones and ki == nK - 1))
                        if with_ones:
                            for mi in range(n_mm):
                                c0, c1 = mi * 512, min(CO, mi * 512 + 512)
                                nc.tensor.matmul(
                                    hp[:, c0:c1], ones1[:], wbot[:, c0:c1],
                                    start=False, stop=True)
                        COH = CO - 6
                        hs = sp.tile([128, COH + 12], f16, tag="hs")
                        if nt % 2 == 0:
                            nc.vector.tensor_copy(hs[:, :COH], hp[:, :COH])
                        else:
                            nc.scalar.copy(hs[:, :COH], hp[:, :COH])
                        nc.vector.tensor_copy(
                            hs[:, COH:COH + 12].bitcast(f32), hp[:, COH:CO])
                        nc.vector.tensor_copy(esed_sb[:, nt * 6:(nt + 1) * 6],
                                              hp[:, CO - 6:CO])
                        eng = nc.sync if nt % 2 == 0 else nc.gpsimd
                        eng.dma_start(
                            out=tbl[nt * 128:(nt + 1) * 128, 0:COH + 12],
                            in_=hs[:])

            # =====================================================
            def aggregate(tbl, ROW, C, esed_sb, esed_dyn, yT, tagp):
                NB = (H * C) // 128
                nbh = C // 128
                with tc.tile_pool(name=f"g{tagp}", bufs=4) as gp, \
                     tc.tile_pool(name=f"o{tagp}", bufs=4) as op, \
                     tc.tile_pool(name=f"a{tagp}", bufs=4) as ap, \
                     tc.tile_pool(name=f"q{tagp}", bufs=2, space="PSUM") as qa, \
                     tc.tile_pool(name=f"e{tagp}", bufs=2, space="PSUM") as qb, \
                     tc.tile_pool(name=f"s{tagp}", bufs=2, space="PSUM") as qc:
                    for b in range(NB):
                        nc.vector.memset(yT[b][:], 0.0)
                    APAD = 1024 if NB == 6 else 512
                    for k in range(CHUNKS):
                        T = int(tiles_per_chunk[k])
                        if T == 0:
                            continue
                        t0 = int(tile_base[k])
                        edch = ap.tile([128, 6], f32, tag="edch")
                        nc.vector.tensor_copy(
                            edch[:], esed_sb[:, k * 6:k * 6 + 6])
                        hg = gp.tile([128, TMAX, ROW], f16, tag="hg")
                        nidx = T * 128
                        nc.gpsimd.dma_gather(
                            hg[:, :T, :], tbl[:], src16[:, t0 * 8:(t0 + T) * 8],
                            nidx, nidx, ROW, single_packet=False)
                        o01t = op.tile([128, TMAX * 128], f16, tag="o01t")
                        oTt = op.tile([128, TMAX * 128], f16, tag="oTt")
                        nc.sync.dma_start(
                            out=o01t[:, :T * 128],
                            in_=o01_d[:, t0 * 128:(t0 + T) * 128])
                        nc.sync.dma_start(
                            out=oTt[:, :T * 128],
                            in_=oT_d[:, t0 * 128:(t0 + T) * 128])

                        # pass 1: ed[dst] per edge -> scores -> exp.
                        # f32 values ride through f16 PE selections as
                        # hi/lo f16 pairs (exact to ~16 mantissa bits).
                        edhl = ap.tile([128, 3], f16, tag="edhl")
                        nc.vector.tensor_copy(edhl[:], edch[:, 3:6])
                        edpA = qb.tile([128, TMAX, 3], f32, tag="edpA")
                        for t in range(T):
                            nc.tensor.matmul(
                                edpA[:, t, :], oTt[:, t * 128:(t + 1) * 128],
                                edhl[:], start=True, stop=True)
                        exf = ap.tile([128, TMAX, 3], f32, tag="exf")
                        nc.vector.tensor_tensor(
                            exf[:, :T, :],
                            hg[:, :T, H * C:H * C + 12].bitcast(f32)[:, :, 0:3],
                            edpA[:, :T, :], OP.add)
                        exm = ap.tile([128, TMAX, 3], f32, tag="exm")
                        nc.vector.tensor_scalar_mul(exm[:, :T, :],
                                                    exf[:, :T, :], NEG_SLOPE)
                        nc.vector.tensor_tensor(exf[:, :T, :], exf[:, :T, :],
                                                exm[:, :T, :], OP.max)
                        nc.vector.tensor_scalar_add(exf[:, :T, :],
                                                    exf[:, :T, :], -5.0)
                        expf = ap.tile([128, TMAX, 3], f32, tag="expf")
                        nc.scalar.activation(expf[:, :T, :], exf[:, :T, :],
                                             AT.Exp)
                        exb = ap.tile([128, TMAX, 3], f16, tag="exb")
                        nc.vector.tensor_copy(exb[:, :T, :], expf[:, :T, :])

                        # pass 2: per-dst sums -> reciprocal -> back to edges
                        spT = qc.tile([128, 4], f32, tag="spT")
                        for t in range(T):
                            nc.tensor.matmul(
                                spT[:, 0:3], o01t[:, t * 128:(t + 1) * 128],
                                exb[:, t, :], start=(t == 0), stop=(t == T - 1))
                        srT = ap.tile([128, 3], f32, tag="srT")
                        nc.vector.tensor_copy(srT[:], spT[:, 0:3])
                        nc.vector.tensor_scalar_add(srT[:], srT[:], 1e-16)
                        nc.vector.reciprocal(srT[:], srT[:])
                        nc.vector.tensor_scalar_min(srT[:], srT[:], 60000.0)
                        srhl = ap.tile([128, 3], f16, tag="srhl")
                        nc.vector.tensor_copy(srhl[:], srT[:])
                        srbA = qb.tile([128, TMAX, 3], f32, tag="edpA")
                        for t in range(T):
                            nc.tensor.matmul(
                                srbA[:, t, :], oTt[:, t * 128:(t + 1) * 128],
                                srhl[:], start=True, stop=True)
                        exsc = ap.tile([128, TMAX, 3], f32, tag="exsc")
                        nc.vector.tensor_tensor(exsc[:, :T, :], expf[:, :T, :],
                                                srbA[:, :T, :], OP.mult)

                        # pass 3: weighted scatter-add via PE
                        aggp = qa.tile([128, APAD], f32, tag="aggp")
                        for t in range(T):
                            ows = []
                            for h in range(H):
                                ow = ap.tile([128, 128], f16, tag=f"ow{h}")
                                if h == 0:
                                    nc.scalar.activation(
                                        ow[:], o01t[:, t * 128:(t + 1) * 128],
                                        AT.Identity,
                                        scale=exsc[:, t, h:h + 1])
                                else:
                                    nc.vector.tensor_scalar(
                                        ow[:], o01t[:, t * 128:(t + 1) * 128],
                                        exsc[:, t, h:h + 1], None, OP.mult)
                                ows.append(ow)
                            for fb in range(NB):
                                h = fb // nbh
                                st = (t == 0) and (fb % 4 == 0)
                                sp2 = (t == T - 1) and (
                                    fb % 4 == 3 or fb == NB - 1)
                                nc.tensor.matmul(
                                    aggp[:, fb * 128:(fb + 1) * 128],
                                    hg[:, t,
                                       h * C + (fb % nbh) * 128:
                                       h * C + (fb % nbh) * 128 + 128],
                                    ows[h][:],
                                    start=st, stop=sp2)
                        # epilogue: relu straight out of PSUM
                        for fb in range(NB):
                            if fb % 3 != 2:
                                nc.scalar.activation(
                                    yT[fb][:, k * 128:(k + 1) * 128],
                                    aggp[:, fb * 128:(fb + 1) * 128], AT.Relu)
                            else:
                                nc.vector.tensor_scalar_max(
                                    yT[fb][:, k * 128:(k + 1) * 128],
                                    aggp[:, fb * 128:(fb + 1) * 128], 0.0)

            # =====================================================
            def bn(yT, NB, bn_d, stl, stf, tagp, apply=True, sc_out=None,
                   sh_out=None):
                with tc.tile_pool(name=f"b{tagp}", bufs=1) as bp:
                    scratch = bp.tile([128, PAD_N], f16, tag="scr")
                    stats = bp.tile([128, 12], f32, tag="stats")
                    nc.vector.memset(stats[:], 0.0)
                    for b in range(NB):
                        nc.vector.tensor_reduce(stats[:, b:b + 1], yT[b][:],
                                                AX.X, OP.add)
                        nc.vector.tensor_tensor(scratch[:], yT[b][:],
                                                yT[b][:], OP.mult)
                        nc.vector.tensor_reduce(stats[:, 6 + b:7 + b],
                                                scratch[:], AX.X, OP.add)
                    nc.sync.dma_start(out=stl[:], in_=stats[:])
                    nc.gpsimd.collective_compute(
                        "AllReduce", OP.add,
                        replica_groups=[list(range(NCORES))],
                        ins=[stl[:]], outs=[stf[:]])
                    stg = bp.tile([128, 12], f32, tag="stg")
                    nc.sync.dma_start(out=stg[:], in_=stf[:])
                    bnp = bp.tile([128, 2 * NB], f32, tag="bnp")
                    nc.sync.dma_start(out=bnp[:], in_=bn_d[:])
                    mu = bp.tile([128, 6], f32, tag="mu")
                    var = bp.tile([128, 6], f32, tag="var")
                    sc = bp.tile([128, 6], f32, tag="sc")
                    sh = bp.tile([128, 6], f32, tag="sh")
                    inv_n = 1.0 / float(N_REAL)
                    nc.vector.tensor_scalar_mul(mu[:, :NB], stg[:, :NB], inv_n)
                    nc.vector.tensor_scalar_mul(var[:, :NB], stg[:, 6:6 + NB],
                                                inv_n)
                    nc.vector.tensor_tensor(sc[:, :NB], mu[:, :NB], mu[:, :NB],
                                            OP.mult)
                    nc.vector.tensor_tensor(var[:, :NB], var[:, :NB],
                                            sc[:, :NB], OP.subtract)
                    nc.vector.tensor_scalar_add(var[:, :NB], var[:, :NB],
                                                BN_EPS)
                    nc.scalar.activation(var[:, :NB], var[:, :NB], AT.Sqrt)
                    nc.vector.reciprocal(var[:, :NB], var[:, :NB])
                    nc.vector.tensor_tensor(sc[:, :NB], bnp[:, :NB],
                                            var[:, :NB], OP.mult)
                    nc.vector.tensor_tensor(mu[:, :NB], mu[:, :NB], sc[:, :NB],
                                            OP.mult)
                    nc.vector.tensor_tensor(sh[:, :NB], bnp[:, NB:2 * NB],
                                            mu[:, :NB], OP.subtract)
                    if apply:
                        for b in range(NB):
                            nc.scalar.activation(yT[b][:], yT[b][:],
                                                 AT.Identity,
                                                 bias=sh[:, b:b + 1],
                                                 scale=sc[:, b:b + 1])
                    else:
                        nc.vector.tensor_copy(sc_out[:], sc[:, :NB])
                        nc.vector.tensor_copy(sh_out[:], sh[:, :NB])

            # ================= Layer 1 =================
            kdims1 = []
            rem = F_IN + 1
            while rem > 0:
                kdims1.append(min(128, rem))
                rem -= kdims1[-1]
            phase1(None, kdims1, W1c_d, F_IN + 1, CO1, h1_loc, LNT, esed1,
                   with_ones=False, tagp="l1")
            nc.gpsimd.collective_compute(
                "AllGather", OP.bypass,
                replica_groups=[list(range(NCORES))],
                ins=[h1_loc[:]], outs=[h1_full[:]])

            def dbg_finish(t128):
                o = pers.tile([1, GPC], f32, tag="dbgy")
                nc.vector.tensor_copy(o[:], t128[0:1, 0:GPC])
                nc.sync.dma_start(out=y_d[:], in_=o[:])

            if STAGE <= 0:
                dbg_finish(esed1)
                return
            with tc.tile_pool(name="y1", bufs=1) as y1p:
                yT1 = [y1p.tile([128, PAD_N], f16, tag=f"y1_{b}", name=f"y1_{b}")
                       for b in range(6)]
                aggregate(h1_full, ROW1, 256, esed1, False, yT1, "l1")
                if STAGE <= 1:
                    dbg_finish(yT1[0])
                    return
                bn(yT1, 6, bn1_d, st_loc[0], st_full[0], "l1")
                if STAGE <= 2:
                    dbg_finish(yT1[0])
                    return

                # ================= Layer 2 =================
                phase1(yT1, [128] * 6, W2c_d, 769, CO23, h2_loc, LNT, esed23,
                       with_ones=True, tagp="l2")
            if STAGE <= 3:
                dbg_finish(esed23)
                return
            nc.gpsimd.collective_compute(
                "AllGather", OP.bypass,
                replica_groups=[list(range(NCORES))],
                ins=[h2_loc[:]], outs=[h2_full[:]])
            if STAGE <= 4:
                dbg_finish(esed23)
                return
            with tc.tile_pool(name="y2", bufs=1) as y2p:
                yT2 = [y2p.tile([128, PAD_N], f16, tag=f"y2_{b}", name=f"y2_{b}")
                       for b in range(3)]
                aggregate(h2_full, ROW23, 128, esed23, False, yT2, "l2")
                if STAGE <= 5:
                    dbg_finish(yT2[0])
                    return
                bn(yT2, 3, bn2_d, st_loc[1], st_full[1], "l2")

                # ================= Layer 3 =================
                phase1(yT2, [128] * 3, W3c_d, 385, CO23, h3_loc, LNT, esed23,
                       with_ones=True, tagp="l3")
            nc.gpsimd.collective_compute(
                "AllGather", OP.bypass,
                replica_groups=[list(range(NCORES))],
                ins=[h3_loc[:]], outs=[h3_full[:]])
            if STAGE <= 7:
                dbg_finish(esed23)
                return
            with tc.tile_pool(name="y3", bufs=1) as y3p:
                yT3 = [y3p.tile([128, PAD_N], f16, tag=f"y3_{b}", name=f"y3_{b}")
                       for b in range(3)]
                # ================= Head =================
                with tc.tile_pool(name="hd", bufs=1) as hp_:
                    sc3 = hp_.tile([128, 3], f32, tag="sc3")
                    sh3 = hp_.tile([128, 3], f32, tag="sh3")
                    aggregate(h3_full, ROW23, 128, esed23, False, yT3, "l3")
                    if STAGE <= 8:
                        dbg_finish(yT3[0])
                        return
                    bn(yT3, 3, bn3_d, st_loc[2], st_full[2], "l3",
                       apply=False, sc_out=sc3, sh_out=sh3)
                    pp_cm = tc.tile_pool(name="hdp", bufs=1, space="PSUM")
                    pp_ = pp_cm.__enter__()
                    cntb = hp_.tile([128, GPC], f32, tag="cntb")
                    nc.sync.dma_start(out=cntb[:], in_=cntb_d[:])
                    poolT = hp_.tile([128, 3 * GPC], f32, tag="poolT")
                    shc = hp_.tile([128, GPC], f32, tag="shc")
                    for b in range(3):
                        for g_ in range(GPC):
                            nc.vector.tensor_reduce(
                                poolT[:, b * GPC + g_:b * GPC + g_ + 1],
                                yT3[b][:, g_ * GST:(g_ + 1) * GST],
                                AX.X, OP.add)
                        # pool(BN(y)) = pool(y)*sc + cnt_g*sh
                        nc.vector.tensor_scalar(
                            poolT[:, b * GPC:(b + 1) * GPC],
                            poolT[:, b * GPC:(b + 1) * GPC],
                            sc3[:, b:b + 1], None, OP.mult)
                        nc.vector.tensor_scalar(
                            shc[:], cntb[:], sh3[:, b:b + 1], None, OP.mult)
                        nc.vector.tensor_tensor(
                            poolT[:, b * GPC:(b + 1) * GPC],
                            poolT[:, b * GPC:(b + 1) * GPC],
                            shc[:], OP.add)
                    fc1w = hp_.tile([128, 3, 64], f32, tag="fc1w")
                    for b in range(3):
                        nc.sync.dma_start(out=fc1w[:, b, :],
                                          in_=fc1w_d[b * 128:(b + 1) * 128, :])
                    fc2w = hp_.tile([64, 64], f32, tag="fc2w")
                    nc.sync.dma_start(out=fc2w[:], in_=fc2w_d[:])
                    fc3w = hp_.tile([64, 1], f32, tag="fc3w")
                    nc.sync.dma_start(out=fc3w[:], in_=fc3w_d[:])
                    fcb = hp_.tile([64, 3], f32, tag="fcb")
                    nc.sync.dma_start(out=fcb[:], in_=fcb_d[:])
                    bn4 = hp_.tile([64, 2], f32, tag="bn4")
                    nc.sync.dma_start(out=bn4[:], in_=bn4_d[:])

                    p1p = pp_.tile([64, GPC], f32, tag="p1p")
                    for b in range(3):
                        nc.tensor.matmul(p1p[:], fc1w[:, b, :],
                                         poolT[:, b * GPC:(b + 1) * GPC],
                                         start=(b == 0), stop=(b == 2))
                    p1 = hp_.tile([64, GPC], f32, tag="p1")
                    nc.scalar.activation(p1[:], p1p[:], AT.Relu,
                                         bias=fcb[:, 0:1])
                    st4 = hp_.tile([64, 2], f32, tag="st4")
                    scr4 = hp_.tile([64, GPC], f32, tag="scr4")
                    nc.vector.tensor_reduce(st4[:, 0:1], p1[:], AX.X, OP.add)
                    nc.vector.tensor_tensor(scr4[:], p1[:], p1[:], OP.mult)
                    nc.vector.tensor_reduce(st4[:, 1:2], scr4[:], AX.X,
                                            OP.add)
                    nc.sync.dma_start(out=st4_loc[:], in_=st4[:])
                    nc.gpsimd.collective_compute(
                        "AllReduce", OP.add,
                        replica_groups=[list(range(NCORES))],
                        ins=[st4_loc[:]], outs=[st4_full[:]])
                    st4g = hp_.tile([64, 2], f32, tag="st4g")
                    nc.sync.dma_start(out=st4g[:], in_=st4_full[:])
                    mu4 = hp_.tile([64, 4], f32, tag="mu4")
                    inv_g = 1.0 / float(G)
                    nc.vector.tensor_scalar_mul(mu4[:, 0:2], st4g[:], inv_g)
                    nc.vector.tensor_tensor(mu4[:, 2:3], mu4[:, 0:1],
                                            mu4[:, 0:1], OP.mult)
                    nc.vector.tensor_tensor(mu4[:, 1:2], mu4[:, 1:2],
                                            mu4[:, 2:3], OP.subtract)
                    nc.vector.tensor_scalar_add(mu4[:, 1:2], mu4[:, 1:2],
                                                BN_EPS)
                    nc.scalar.activation(mu4[:, 1:2], mu4[:, 1:2], AT.Sqrt)
                    nc.vector.reciprocal(mu4[:, 1:2], mu4[:, 1:2])
                    nc.vector.tensor_tensor(mu4[:, 2:3], bn4[:, 0:1],
                                            mu4[:, 1:2], OP.mult)
                    nc.vector.tensor_tensor(mu4[:, 3:4], mu4[:, 0:1],
                                            mu4[:, 2:3], OP.mult)
                    nc.vector.tensor_tensor(mu4[:, 3:4], bn4[:, 1:2],
                                            mu4[:, 3:4], OP.subtract)
                    nc.scalar.activation(p1[:], p1[:], AT.Identity,
                                         bias=mu4[:, 3:4], scale=mu4[:, 2:3])
                    p2p = pp_.tile([64, GPC], f32, tag="p2p")
                    nc.tensor.matmul(p2p[:], fc2w[:], p1[:], start=True,
                                     stop=True)
                    p2 = hp_.tile([64, GPC], f32, tag="p2")
                    nc.scalar.activation(p2[:], p2p[:], AT.Relu,
                                         bias=fcb[:, 1:2])
                    p3p = pp_.tile([1, GPC], f32, tag="p3p")
                    nc.tensor.matmul(p3p[:], fc3w[:], p2[:], start=True,
                                     stop=True)
                    p3 = hp_.tile([1, GPC], f32, tag="p3")
                    nc.scalar.activation(p3[:], p3p[:], AT.Relu,
                                         bias=fcb[0:1, 2:3])
                    nc.sync.dma_start(out=y_d[:], in_=p3[:])
                    pp_cm.__exit__(None, None, None)


def make_in_maps(P):
    in_maps = []
    for c in range(NCORES):
        in_maps.append({
            "x1T": P["x1T"][c], "W1c": P["W1c"], "W2c": P["W2c"], "W3c": P["W3c"],
            "src16": P["src16"][c], "o01": P["o01"][c], "oT": P["oT"][c],
            "ones1": P["ones1"],
            "bn1": P["bn1"], "bn2": P["bn2"], "bn3": P["bn3"], "bn4": P["bn4"],
            "fc1w": P["fc1w"], "fc2w": P["fc2w"], "fc3w": P["fc3w"],
            "fcb": P["fcb"], "cntb": P["cntb"][c],
        })
    return in_maps


def build_program(P):
    from concourse import bass, mybir, tile, bacc, library_config
    nc = bacc.Bacc("TRN2", target_bir_lowering=False, debug=False,
                   num_devices=NCORES)
    _build(nc, P, mybir, tile, bass, library_config)
    nc.compile()
    return nc


def kernel(**inputs):
    from concourse.bass_utils import run_bass_kernel_spmd
    P = _prep(inputs)
    nc = build_program(P)
    res = run_bass_kernel_spmd(nc, make_in_maps(P), list(range(NCORES)))
    out = np.concatenate([res.results[c]["y"][0] for c in range(NCORES)])
    return out.astype(np.float32)
